# revision 2
# baseline (speedup 1.0000x reference)
# ISTFT kernel v3: v2 + sim-guided tweaks:
#   - no SWDGE: mag loaded f32 via HWDGE, muls f32*f32 -> bf16 spec
#   - output DMAs consolidated (1 per psum-pair tile for interior tiles) and
#     issued from the ACT HWDGE ring (nc.scalar) to unload the SP sequencer
import numpy as np

import concourse.bacc as bacc
import concourse.mybir as mybir
import concourse.tile as tile
from concourse.bass_utils import run_bass_kernel_spmd

F32 = mybir.dt.float32
BF16 = mybir.dt.bfloat16
ALU = mybir.AluOpType
ACTF = mybir.ActivationFunctionType

TWO_PI = 6.283185307179586
INV_2PI = 1.0 / TWO_PI
MAGIC = 12582912.0
PI = 3.141592653589793
HALF_PI = PI / 2
SIN_SCALE = 0.999999

B_LOCAL = 2
T = 2048
N_CORES = 8


def build_nc(repeat=1):
    nc = bacc.Bacc(target_bir_lowering=False)
    mag = nc.declare_dram_parameter("mag", [B_LOCAL, 513, T], F32, isOutput=False)
    ang = nc.declare_dram_parameter("angle", [B_LOCAL, 513, T], F32, isOutput=False)
    invb = nc.declare_dram_parameter("invbasis", [1026, 1024], F32, isOutput=False)
    out = nc.declare_dram_parameter("out", [B_LOCAL, 523008], F32, isOutput=True)

    with tile.TileContext(nc) as tc:
        with (
            tc.tile_pool(name="const", bufs=1) as constp,
            tc.tile_pool(name="setup", bufs=1) as setupp,
            tc.tile_pool(name="ang", bufs=2) as angp,
            tc.tile_pool(name="magp", bufs=2) as magp,
            tc.tile_pool(name="spec", bufs=2) as specp,
            tc.tile_pool(name="work", bufs=2) as workp,
            tc.tile_pool(name="r512", bufs=2) as r512p,
            tc.tile_pool(name="osb", bufs=4) as osbp,
            tc.tile_pool(name="psum", bufs=8, space="PSUM") as psump,
        ):
            # --- invbasis: load f32 chunks, cast to resident bf16 tiles (one-time) ---
            ib = []
            for q in range(8):
                ibf = setupp.tile([128, 1024], F32, tag="ibf")
                if q < 4:
                    nc.sync.dma_start(out=ibf[:, :], in_=invb[128 * q : 128 * (q + 1), :])
                elif q == 4:
                    nc.sync.dma_start(out=ibf[0:1, :], in_=invb[512:513, :])
                    nc.sync.dma_start(out=ibf[1:128, :], in_=invb[514:641, :])
                else:
                    base = 641 + 128 * (q - 5)
                    nc.sync.dma_start(out=ibf[:, :], in_=invb[base : base + 128, :])
                ibq = constp.tile([128, 1024], BF16, tag=f"ib{q}", name=f"ib{q}")
                nc.vector.tensor_copy(ibq[:, :], ibf[:, :])
                ib.append(ibq)

            for b in [b for _ in range(repeat) for b in range(B_LOCAL)]:
                spec = []
                for q in range(8):
                    st = specp.tile([128, 4 + T], BF16, tag=f"spec{q}", name=f"spec{q}")
                    nc.vector.memset(st[:, 0:4], 0.0)
                    spec.append(st)

                # --- main elementwise: 4 q-chains at full width ---
                for q in range(4):
                    rows = slice(128 * q, 128 * (q + 1))
                    at = angp.tile([128, T], F32, tag="at", name="at")
                    nc.sync.dma_start(out=at[:, :], in_=ang[b, rows, :])
                    mt = magp.tile([128, T], F32, tag="mt", name="mt")
                    nc.sync.dma_start(out=mt[:, :], in_=mag[b, rows, :])

                    tb = workp.tile([128, T], F32, tag="tb", name="tb")
                    red = workp.tile([128, T], F32, tag="red", name="red")
                    sv = workp.tile([128, T], BF16, tag="sv", name="sv")
                    cv = workp.tile([128, T], BF16, tag="cv", name="cv")
                    nc.vector.tensor_scalar(
                        out=tb, in0=at[:, :], scalar1=INV_2PI, scalar2=MAGIC,
                        op0=ALU.mult, op1=ALU.add,
                    )
                    nc.vector.tensor_scalar_sub(tb, tb, MAGIC)
                    nc.vector.scalar_tensor_tensor(
                        out=red, in0=tb, scalar=-TWO_PI, in1=at[:, :],
                        op0=ALU.mult, op1=ALU.add,
                    )
                    nc.vector.add_range_wrap(
                        out=tb, in_=red, shift=HALF_PI, bound=PI, period=TWO_PI
                    )
                    nc.scalar.activation(sv, red, ACTF.Sin, scale=SIN_SCALE)
                    nc.scalar.activation(cv, tb, ACTF.Sin, scale=SIN_SCALE)
                    nc.vector.tensor_mul(spec[4 + q][:, 4 : 4 + T], mt[:, :], sv)
                    nc.vector.tensor_mul(spec[q][:, 4 : 4 + T], mt[:, :], cv)

                # --- row 512 (real only), packed [16, 128] ---
                a5 = r512p.tile([16, 128], F32, tag="a5", name="a5")
                nc.sync.dma_start(out=a5[:, :], in_=ang[b, 512:513, :])
                m5 = r512p.tile([16, 128], F32, tag="m5", name="m5")
                nc.sync.dma_start(out=m5[:, :], in_=mag[b, 512:513, :])
                t5 = r512p.tile([16, 128], F32, tag="t5", name="t5")
                r5 = r512p.tile([16, 128], F32, tag="r5", name="r5")
                c5 = r512p.tile([16, 128], BF16, tag="c5", name="c5")
                p5 = r512p.tile([16, 128], BF16, tag="p5", name="p5")
                nc.vector.tensor_scalar(
                    out=t5, in0=a5[:, :], scalar1=INV_2PI, scalar2=0.25,
                    op0=ALU.mult, op1=ALU.add,
                )
                nc.vector.tensor_scalar(
                    out=t5, in0=t5, scalar1=1.0, scalar2=MAGIC,
                    op0=ALU.mult, op1=ALU.add,
                )
                nc.vector.tensor_scalar_sub(t5, t5, MAGIC)
                nc.vector.scalar_tensor_tensor(
                    out=r5, in0=t5, scalar=-TWO_PI, in1=a5[:, :],
                    op0=ALU.mult, op1=ALU.add,
                )
                nc.vector.add_range_wrap(
                    out=t5, in_=r5, shift=HALF_PI, bound=PI, period=TWO_PI
                )
                nc.scalar.activation(c5, t5, ACTF.Sin, scale=SIN_SCALE)
                nc.vector.tensor_mul(p5, m5[:, :], c5)
                nc.sync.dma_start(out=spec[4][0:1, 4 : 4 + T], in_=p5[:, :])

                # --- matmuls: 8 full-bank psum tiles, each = ct pair ---
                for pt in range(8):
                    ps = psump.tile([128, 512], F32, tag="ps", name="ps")
                    for half in range(2):
                        ct = 2 * pt + half
                        c0 = 128 * ct
                        mmi = 0
                        for q in range(8):
                            for j in range(4):
                                nc.tensor.matmul(
                                    out=ps[:, 256 * half : 256 * half + 256],
                                    lhsT=spec[q][:, c0 - j + 4 : c0 - j + 132],
                                    rhs=ib[q][:, 256 * j : 256 * (j + 1)],
                                    start=(mmi == 0),
                                    stop=(mmi == 31),
                                )
                                mmi += 1
                    ob = osbp.tile([128, 512], F32, tag="ob", name="ob")
                    nc.scalar.activation(ob[:, :], ps[:, :], ACTF.Copy)
                    if pt == 0:
                        nc.scalar.dma_start(out=out[b, 0:31744], in_=ob[4:128, 0:256])
                        nc.scalar.dma_start(
                            out=out[b, 31744:64512], in_=ob[:, 256:512]
                        )
                    elif pt == 7:
                        lo = 256 * (128 * 14 - 4)
                        nc.scalar.dma_start(
                            out=out[b, lo : lo + 32768], in_=ob[:, 0:256]
                        )
                        lo2 = 256 * (128 * 15 - 4)
                        nc.scalar.dma_start(
                            out=out[b, lo2 : lo2 + 32512], in_=ob[0:127, 256:512]
                        )
                    else:
                        lo = 256 * (256 * pt - 4)
                        nc.scalar.dma_start(
                            out=out[b, lo : lo + 32768], in_=ob[:, 0:256]
                        )
                        nc.scalar.dma_start(
                            out=out[b, lo + 32768 : lo + 65536], in_=ob[:, 256:512]
                        )
    nc.compile()
    return nc


_CACHE = {}


def _get_nc():
    if "nc" not in _CACHE:
        _CACHE["nc"] = build_nc()
    return _CACHE["nc"]


def kernel(mag, angle, invbasis, _trace=False, **_ignored):
    nc = _get_nc()
    mag = np.ascontiguousarray(np.asarray(mag, dtype=np.float32))
    angle = np.ascontiguousarray(np.asarray(angle, dtype=np.float32))
    invbasis = np.ascontiguousarray(np.asarray(invbasis, dtype=np.float32))
    in_maps = [
        {
            "mag": mag[B_LOCAL * i : B_LOCAL * (i + 1)],
            "angle": angle[B_LOCAL * i : B_LOCAL * (i + 1)],
            "invbasis": invbasis,
        }
        for i in range(N_CORES)
    ]
    res = run_bass_kernel_spmd(nc, in_maps, list(range(N_CORES)), trace=_trace)
    outs = [res.results[i]["out"] for i in range(N_CORES)]
    full = np.concatenate(outs, axis=0).reshape(16, 1, 523008)
    if _trace:
        return full, res
    return full


# revision 3
# speedup vs baseline: 2.5606x; 2.5606x over previous
# ISTFT kernel for Trainium2 (8 NeuronCores, data-parallel over batch).
#
# Math: out[256*c + r] = sum_{j=0..3} sum_C spec[C, c-j] * invbasis[C, 256*j + r],
# i.e. the ConvTranspose1d overlap-add is folded into 4 shifted matmuls that
# accumulate in PSUM. invbasis rows 513 (imag DC) and 1025 (imag Nyquist) are
# exactly zero, so the contraction packs to 1024 rows = 8 blocks of 128:
#   packed rows 0..512    = real rows 0..512   (mag*cos(angle))
#   packed rows 513..1023 = imag freqs 1..511  (mag*sin(angle))
# Output keeps chunks 4..2046 (reference trims NFFT=1024 samples per side).
#
# Structure (per core, 2 batches):
#   - inputs loaded as 4x [128, 2048] f32 tiles each for angle/mag (big HWDGE
#     DMAs); row 512 handled separately as a packed [16, 128] tile whose
#     product is scattered into spec[4] partition 0 by an SBUF->SBUF DMA
#   - elementwise on full-width tiles: magic-round range reduction (DVE),
#     sin/cos via ACT Sin (table domain [-pi, pi]), muls write bf16 spec tiles
#   - matmuls: 8 PSUM tiles of [128, 512] f32 — one FULL 2KB bank each (two
#     256-wide output chunks share a bank; half-bank tiles would serialize the
#     ACT evacuation against PE accumulation on the shared bank)
#   - PSUM evacuated by ACT Copy; output DMAs issued from the ACT HWDGE ring
#     (nc.scalar) so the SP sequencer only carries the input loads
import numpy as np

import concourse.bacc as bacc
import concourse.mybir as mybir
import concourse.tile as tile
from concourse.bass_utils import run_bass_kernel_spmd

F32 = mybir.dt.float32
BF16 = mybir.dt.bfloat16
ALU = mybir.AluOpType
ACTF = mybir.ActivationFunctionType

TWO_PI = 6.283185307179586
INV_2PI = 1.0 / TWO_PI
MAGIC = 12582912.0
PI = 3.141592653589793
HALF_PI = PI / 2
SIN_SCALE = 0.999999

B_LOCAL = 2
T = 2048
N_CORES = 8


def build_nc(repeat=1):
    nc = bacc.Bacc(target_bir_lowering=False)
    mag = nc.declare_dram_parameter("mag", [B_LOCAL, 513, T], F32, isOutput=False)
    ang = nc.declare_dram_parameter("angle", [B_LOCAL, 513, T], F32, isOutput=False)
    invb = nc.declare_dram_parameter("invbasis", [1026, 1024], F32, isOutput=False)
    out = nc.declare_dram_parameter("out", [B_LOCAL, 523008], F32, isOutput=True)

    with tile.TileContext(nc) as tc:
        with (
            tc.tile_pool(name="const", bufs=1) as constp,
            tc.tile_pool(name="setup", bufs=1) as setupp,
            tc.tile_pool(name="ang", bufs=2) as angp,
            tc.tile_pool(name="magp", bufs=2) as magp,
            tc.tile_pool(name="spec", bufs=2) as specp,
            tc.tile_pool(name="work", bufs=2) as workp,
            tc.tile_pool(name="r512", bufs=2) as r512p,
            tc.tile_pool(name="osb", bufs=4) as osbp,
            tc.tile_pool(name="psum", bufs=8, space="PSUM") as psump,
        ):
            # --- invbasis: load f32 chunks, cast to resident bf16 tiles (one-time) ---
            ib = []
            for q in range(8):
                ibf = setupp.tile([128, 1024], F32, tag="ibf")
                if q < 4:
                    nc.sync.dma_start(out=ibf[:, :], in_=invb[128 * q : 128 * (q + 1), :])
                elif q == 4:
                    nc.sync.dma_start(out=ibf[0:1, :], in_=invb[512:513, :])
                    nc.sync.dma_start(out=ibf[1:128, :], in_=invb[514:641, :])
                else:
                    base = 641 + 128 * (q - 5)
                    nc.sync.dma_start(out=ibf[:, :], in_=invb[base : base + 128, :])
                ibq = constp.tile([128, 1024], BF16, tag=f"ib{q}", name=f"ib{q}")
                nc.vector.tensor_copy(ibq[:, :], ibf[:, :])
                ib.append(ibq)

            for b in [b for _ in range(repeat) for b in range(B_LOCAL)]:
                spec = []
                for q in range(8):
                    st = specp.tile([128, 4 + T], BF16, tag=f"spec{q}", name=f"spec{q}")
                    nc.vector.memset(st[:, 0:4], 0.0)
                    spec.append(st)

                # --- main elementwise: 4 q-chains at full width ---
                for q in range(4):
                    rows = slice(128 * q, 128 * (q + 1))
                    at = angp.tile([128, T], F32, tag="at", name="at")
                    nc.sync.dma_start(out=at[:, :], in_=ang[b, rows, :])
                    mt = magp.tile([128, T], F32, tag="mt", name="mt")
                    nc.sync.dma_start(out=mt[:, :], in_=mag[b, rows, :])

                    tb = workp.tile([128, T], F32, tag="tb", name="tb")
                    red = workp.tile([128, T], F32, tag="red", name="red")
                    sv = workp.tile([128, T], BF16, tag="sv", name="sv")
                    cv = workp.tile([128, T], BF16, tag="cv", name="cv")
                    nc.vector.tensor_scalar(
                        out=tb, in0=at[:, :], scalar1=INV_2PI, scalar2=MAGIC,
                        op0=ALU.mult, op1=ALU.add,
                    )
                    nc.vector.tensor_scalar_sub(tb, tb, MAGIC)
                    nc.vector.scalar_tensor_tensor(
                        out=red, in0=tb, scalar=-TWO_PI, in1=at[:, :],
                        op0=ALU.mult, op1=ALU.add,
                    )
                    nc.vector.add_range_wrap(
                        out=tb, in_=red, shift=HALF_PI, bound=PI, period=TWO_PI
                    )
                    nc.scalar.activation(sv, red, ACTF.Sin, scale=SIN_SCALE)
                    nc.scalar.activation(cv, tb, ACTF.Sin, scale=SIN_SCALE)
                    nc.vector.tensor_mul(spec[4 + q][:, 4 : 4 + T], mt[:, :], sv)
                    nc.vector.tensor_mul(spec[q][:, 4 : 4 + T], mt[:, :], cv)

                # --- row 512 (real only), packed [16, 128] ---
                a5 = r512p.tile([16, 128], F32, tag="a5", name="a5")
                nc.sync.dma_start(out=a5[:, :], in_=ang[b, 512:513, :])
                m5 = r512p.tile([16, 128], F32, tag="m5", name="m5")
                nc.sync.dma_start(out=m5[:, :], in_=mag[b, 512:513, :])
                t5 = r512p.tile([16, 128], F32, tag="t5", name="t5")
                r5 = r512p.tile([16, 128], F32, tag="r5", name="r5")
                c5 = r512p.tile([16, 128], BF16, tag="c5", name="c5")
                p5 = r512p.tile([16, 128], BF16, tag="p5", name="p5")
                nc.vector.tensor_scalar(
                    out=t5, in0=a5[:, :], scalar1=INV_2PI, scalar2=0.25,
                    op0=ALU.mult, op1=ALU.add,
                )
                nc.vector.tensor_scalar(
                    out=t5, in0=t5, scalar1=1.0, scalar2=MAGIC,
                    op0=ALU.mult, op1=ALU.add,
                )
                nc.vector.tensor_scalar_sub(t5, t5, MAGIC)
                nc.vector.scalar_tensor_tensor(
                    out=r5, in0=t5, scalar=-TWO_PI, in1=a5[:, :],
                    op0=ALU.mult, op1=ALU.add,
                )
                nc.vector.add_range_wrap(
                    out=t5, in_=r5, shift=HALF_PI, bound=PI, period=TWO_PI
                )
                nc.scalar.activation(c5, t5, ACTF.Sin, scale=SIN_SCALE)
                nc.vector.tensor_mul(p5, m5[:, :], c5)
                nc.sync.dma_start(out=spec[4][0:1, 4 : 4 + T], in_=p5[:, :])

                # --- matmuls: 8 full-bank psum tiles, each = ct pair ---
                for pt in range(8):
                    ps = psump.tile([128, 512], F32, tag="ps", name="ps")
                    for half in range(2):
                        ct = 2 * pt + half
                        c0 = 128 * ct
                        mmi = 0
                        for q in range(8):
                            for j in range(4):
                                nc.tensor.matmul(
                                    out=ps[:, 256 * half : 256 * half + 256],
                                    lhsT=spec[q][:, c0 - j + 4 : c0 - j + 132],
                                    rhs=ib[q][:, 256 * j : 256 * (j + 1)],
                                    start=(mmi == 0),
                                    stop=(mmi == 31),
                                )
                                mmi += 1
                    ob = osbp.tile([128, 512], F32, tag="ob", name="ob")
                    nc.scalar.activation(ob[:, :], ps[:, :], ACTF.Copy)
                    if pt == 0:
                        nc.scalar.dma_start(out=out[b, 0:31744], in_=ob[4:128, 0:256])
                        nc.scalar.dma_start(
                            out=out[b, 31744:64512], in_=ob[:, 256:512]
                        )
                    elif pt == 7:
                        lo = 256 * (128 * 14 - 4)
                        nc.scalar.dma_start(
                            out=out[b, lo : lo + 32768], in_=ob[:, 0:256]
                        )
                        lo2 = 256 * (128 * 15 - 4)
                        nc.scalar.dma_start(
                            out=out[b, lo2 : lo2 + 32512], in_=ob[0:127, 256:512]
                        )
                    else:
                        lo = 256 * (256 * pt - 4)
                        nc.scalar.dma_start(
                            out=out[b, lo : lo + 32768], in_=ob[:, 0:256]
                        )
                        nc.scalar.dma_start(
                            out=out[b, lo + 32768 : lo + 65536], in_=ob[:, 256:512]
                        )
    nc.compile()
    return nc


_CACHE = {}


def _get_nc():
    if "nc" not in _CACHE:
        _CACHE["nc"] = build_nc()
    return _CACHE["nc"]


def kernel(mag, angle, invbasis, _trace=False, **_ignored):
    nc = _get_nc()
    mag = np.ascontiguousarray(np.asarray(mag, dtype=np.float32))
    angle = np.ascontiguousarray(np.asarray(angle, dtype=np.float32))
    invbasis = np.ascontiguousarray(np.asarray(invbasis, dtype=np.float32))
    in_maps = [
        {
            "mag": mag[B_LOCAL * i : B_LOCAL * (i + 1)],
            "angle": angle[B_LOCAL * i : B_LOCAL * (i + 1)],
            "invbasis": invbasis,
        }
        for i in range(N_CORES)
    ]
    res = run_bass_kernel_spmd(nc, in_maps, list(range(N_CORES)), trace=_trace)
    outs = [res.results[i]["out"] for i in range(N_CORES)]
    full = np.concatenate(outs, axis=0).reshape(16, 1, 523008)
    if _trace:
        return full, res
    return full
